# revision 25
# baseline (speedup 1.0000x reference)
"""Trainium2 Bass kernel for EuclideanSimilarity:
out[i, j] = -||z_anc[i] - z_pos_neg[j]||_2
          = -sqrt(a2[i] + b2[j] - 2 * z_anc[i] . z_pos_neg[j])

Sharding: z_anc rows split across 8 cores (1024 rows each); z_pos_neg
replicated.  Each core computes a [1024, 8192] slab of the output.

Per-core device program (engine balance: ACT ~63us is the floor, PE
~56us, DVE ~28us, DMA ~48us):
  - ab via TensorE: bf16 K=128 matmuls, lhsT = aT block [128d x 128m]
    stationary, rhs = bT [128d x 512n] moving.
  - b2 folded into the same PSUM accumulation via a SECOND K=128 matmul
    that consumes the elementwise squares directly:
      psum += (-0.5*ones[128,128])^T @ sq   where sq[d,n] = bT[d,n]^2
    i.e. the partition reduction (b2[n] = sum_d b[d,n]^2), the broadcast
    and the -0.5 scale all happen inside one matmul -- no separate
    reduction pipeline, no extra PSUM.  All-K=128 matmuls keep the PE at
    its ~220ns/MM pipelined cadence (K=2 rank-1 updates measured 2x
    slower due to LDW/pipe interaction).
  - a2 via DVE only: square + free-dim reduce on the natural-layout
    slice aN [1024 x 128], giving per-partition bias columns directly.
  - ACT: sqrt(-2*psum + a2[m]) -> fp16, one FD=2048 pass per psum tile
    (ACT costs 261ns + 0.833ns/elem regardless of func/bias/dtype, so
    wide tiles amortize the fixed cost; this engine is the bottleneck).
  - DVE negate (fp16 4x mode), DMA out [128, 2048] fp16 tiles.
"""

import os
import sys

import numpy as np
import ml_dtypes

try:
    import concourse  # noqa: F401
except ImportError:
    for _p in ("/opt/trn_rl_repo", os.path.expanduser("~/.axon_site/_ro/trn_rl_repo")):
        if os.path.isdir(_p) and _p not in sys.path:
            sys.path.insert(0, _p)

import concourse.bass as bass  # noqa: F401
import concourse.mybir as mybir
import concourse.tile as tile
from concourse import bacc
from concourse import bass_utils

N_CORES = 8
N, M, D = 8192, 8192, 128
R = N // N_CORES  # 1024 rows of z_anc per core
P = 128           # partitions
BANK = 512        # fp32 columns per PSUM bank
GRP = 2048        # columns per ACT/DVE/DMA group (4 banks)
MT = R // P       # 8 m-tiles per core
NG = M // GRP     # 4 n-groups

OUT_DT = mybir.dt.float16
_BF16 = ml_dtypes.bfloat16

_nc_cache = None


def _build():
    f32 = mybir.dt.float32
    bf16 = mybir.dt.bfloat16

    nc = bacc.Bacc("TRN2", debug=False, target_bir_lowering=False)
    aT = nc.dram_tensor("aT", [P, R], bf16, kind="ExternalInput").ap()
    aN = nc.dram_tensor("aN", [R, P], bf16, kind="ExternalInput").ap()
    bT = nc.dram_tensor("bT", [P, M], bf16, kind="ExternalInput").ap()
    out = nc.dram_tensor("out", [R, M], OUT_DT, kind="ExternalOutput").ap()

    with tile.TileContext(nc) as tc:
        with tc.tile_pool(name="consts", bufs=1) as consts:
            bT_sb = consts.tile([P, M], bf16)
            # group-0 columns first, in bank-sized sub-DMAs so the first
            # tile's squares/matmuls start as soon as sub-chunk 0 lands
            nc.sync.dma_start(out=bT_sb[:, 0:BANK], in_=bT[:, 0:BANK])
            aT_sb = consts.tile([P, R], bf16)
            nc.sync.dma_start(out=aT_sb, in_=aT)
            for j in range(1, GRP // BANK):
                sl = slice(j * BANK, (j + 1) * BANK)
                nc.sync.dma_start(out=bT_sb[:, sl], in_=bT[:, sl])
            # all eight aN row-tiles in ONE DMA via a 3D access pattern
            aN8 = consts.tile([P, R], bf16)  # [p, (t d)]
            aN_r = bass.AP(
                tensor=aN.tensor, offset=aN.offset,
                ap=[[D, P], [P * D, MT], [1, D]],
            )
            nc.sync.dma_start(
                out=aN8.rearrange("p (t d) -> p t d", d=D), in_=aN_r
            )

            w_b2 = consts.tile([P, P], bf16)   # -0.5, turns sq into -b2/2
            nc.vector.memset(w_b2, -0.5)
            scratch = consts.tile([P, BANK], bf16)  # PE warmup fodder
            nc.vector.memset(scratch, 0.001)
            junk = consts.tile([P, 8], f32)
            biasj = consts.tile([P, 1], f32)
            nc.vector.memset(biasj, 1.0)

            sqfull = consts.tile([P, M], bf16)  # bT^2, consumed by b2 matmuls
            a2c = consts.tile([P, MT], f32)     # ACT bias columns (= a2)

            with (
                tc.tile_pool(name="mm", bufs=2, space="PSUM") as mm_pool,
                tc.tile_pool(name="asq", bufs=2) as asq_pool,
                tc.tile_pool(name="o", bufs=6) as o_pool,
                tc.tile_pool(name="on", bufs=6) as on_pool,
            ):
                # preload the sqrt ACT table while DMAs are in flight
                nc.scalar.activation(
                    junk, scratch[:, 0:8], mybir.ActivationFunctionType.Sqrt,
                    bias=biasj[:, 0:1],
                )
                # PE warmup: HAM un-throttles after ~3.4us of activity, so
                # burn the DMA wait keeping the PE busy on scratch data
                wu = mm_pool.tile([P, GRP], f32, tag="ps")
                for k in range(6):
                    nc.tensor.matmul(
                        wu[:, (k % 4) * BANK:(k % 4) * BANK + BANK],
                        lhsT=scratch[:, 0:P], rhs=scratch,
                        start=True, stop=True,
                    )

                # squares for group 0 ahead of the a2 chain (DVE is in-order)
                for j in range(GRP // BANK):
                    sl = slice(j * BANK, (j + 1) * BANK)
                    nc.vector.tensor_mul(sqfull[:, sl], bT_sb[:, sl], bT_sb[:, sl])

                # ---- a2 on DVE only: square + free-dim reduce ----------
                for t in range(MT):
                    sl = slice(t * P, (t + 1) * P)
                    asq = asq_pool.tile([P, P], bf16, tag="asq")
                    nc.vector.tensor_mul(asq, aN8[:, sl], aN8[:, sl])
                    nc.vector.tensor_reduce(
                        a2c[:, t:t + 1], asq,
                        axis=mybir.AxisListType.X, op=mybir.AluOpType.add,
                    )

                # ---- main loop (n-group-major) -------------------------
                for g in range(NG):
                    # fetch the NEXT group's bT columns while computing this
                    if g + 1 < NG:
                        sl = slice((g + 1) * GRP, (g + 2) * GRP)
                        nc.sync.dma_start(out=bT_sb[:, sl], in_=bT[:, sl])
                    for j in range(GRP // BANK if g > 0 else 0):
                        c0 = g * GRP + j * BANK
                        sl = slice(c0, c0 + BANK)
                        nc.vector.tensor_mul(
                            sqfull[:, sl], bT_sb[:, sl], bT_sb[:, sl]
                        )
                    for t in range(MT):
                        ps = mm_pool.tile([P, GRP], f32, tag="ps")
                        if g == 0 and t < 2:
                            # bank-granular ramp: ACT starts after the first
                            # matmul pair instead of a full 4-bank tile
                            for j in range(GRP // BANK):
                                c0 = j * BANK
                                pb = ps[:, c0:c0 + BANK]
                                nc.tensor.matmul(
                                    pb, lhsT=aT_sb[:, t * P:(t + 1) * P],
                                    rhs=bT_sb[:, c0:c0 + BANK],
                                    start=True, stop=False,
                                )
                                nc.tensor.matmul(
                                    pb, lhsT=w_b2, rhs=sqfull[:, c0:c0 + BANK],
                                    start=False, stop=True,
                                )
                                o = o_pool.tile([P, BANK], OUT_DT, tag="o0")
                                nc.scalar.activation(
                                    o, pb, mybir.ActivationFunctionType.Sqrt,
                                    bias=a2c[:, t:t + 1], scale=-2.0,
                                )
                                on = on_pool.tile([P, BANK], OUT_DT, tag="on0")
                                nc.vector.tensor_scalar_mul(on, o, -1.0)
                                nc.sync.dma_start(
                                    out=out[t * P:(t + 1) * P, c0:c0 + BANK],
                                    in_=on,
                                )
                            continue
                        for j in range(GRP // BANK):
                            c0 = g * GRP + j * BANK
                            nc.tensor.matmul(
                                ps[:, j * BANK:(j + 1) * BANK],
                                lhsT=aT_sb[:, t * P:(t + 1) * P],
                                rhs=bT_sb[:, c0:c0 + BANK],
                                start=True, stop=False,
                            )
                        for j in range(GRP // BANK):
                            c0 = g * GRP + j * BANK
                            nc.tensor.matmul(
                                ps[:, j * BANK:(j + 1) * BANK],
                                lhsT=w_b2,
                                rhs=sqfull[:, c0:c0 + BANK],
                                start=False, stop=True,
                            )
                        o = o_pool.tile([P, GRP], OUT_DT, tag="o")
                        nc.scalar.activation(
                            o, ps, mybir.ActivationFunctionType.Sqrt,
                            bias=a2c[:, t:t + 1], scale=-2.0,
                        )
                        on = on_pool.tile([P, GRP], OUT_DT, tag="on")
                        nc.vector.tensor_scalar_mul(on, o, -1.0)
                        nc.sync.dma_start(
                            out=out[t * P:(t + 1) * P, g * GRP:(g + 1) * GRP],
                            in_=on,
                        )

    nc.compile()
    return nc


def _get_nc():
    global _nc_cache
    if _nc_cache is None:
        _nc_cache = _build()
    return _nc_cache


def _in_maps(z_anc, z_pos_neg):
    za = np.asarray(z_anc, dtype=np.float32)
    zaT = np.ascontiguousarray(za.T)
    zbT = np.ascontiguousarray(np.asarray(z_pos_neg, dtype=np.float32).T)
    bT = zbT.astype(_BF16)
    maps = []
    for c in range(N_CORES):
        rows = slice(c * R, (c + 1) * R)
        aTc = np.ascontiguousarray(zaT[:, rows]).astype(_BF16)
        aNc = np.ascontiguousarray(za[rows, :]).astype(_BF16)
        maps.append({"aT": aTc, "aN": aNc, "bT": bT})
    return maps


def run(z_anc, z_pos_neg, **kwargs):
    """Run on hardware; returns (full_output, BassKernelResults)."""
    nc = _get_nc()
    res = bass_utils.run_bass_kernel_spmd(
        nc, _in_maps(z_anc, z_pos_neg), core_ids=list(range(N_CORES)), **kwargs
    )
    out = np.concatenate([r["out"] for r in res.results], axis=0)
    return out.astype(np.float32), res


def kernel(z_anc, z_pos_neg):
    out, _ = run(z_anc, z_pos_neg)
    return out


# revision 26
# speedup vs baseline: 1.0763x; 1.0763x over previous
"""Trainium2 Bass kernel for EuclideanSimilarity:
out[i, j] = -||z_anc[i] - z_pos_neg[j]||_2
          = -sqrt(a2[i] + b2[j] - 2 * z_anc[i] . z_pos_neg[j])

Sharding: z_anc rows split across 8 cores (1024 rows each); z_pos_neg
replicated.  Each core computes a [1024, 8192] slab of the output.

Per-core device program (engine balance: ACT ~63us is the floor, PE
~56us, DVE ~28us, DMA ~48us):
  - ab via TensorE: bf16 K=128 matmuls, lhsT = aT block [128d x 128m]
    stationary, rhs = bT [128d x 512n] moving.
  - b2 folded into the same PSUM accumulation via a SECOND K=128 matmul
    that consumes the elementwise squares directly:
      psum += (-0.5*ones[128,128])^T @ sq   where sq[d,n] = bT[d,n]^2
    i.e. the partition reduction (b2[n] = sum_d b[d,n]^2), the broadcast
    and the -0.5 scale all happen inside one matmul -- no separate
    reduction pipeline, no extra PSUM.  All-K=128 matmuls keep the PE at
    its ~220ns/MM pipelined cadence (K=2 rank-1 updates measured 2x
    slower due to LDW/pipe interaction).
  - a2 via DVE only: square + free-dim reduce on the natural-layout
    slice aN [1024 x 128], giving per-partition bias columns directly.
  - ACT: sqrt(-2*psum + a2[m]) -> fp16, one FD=2048 pass per psum tile
    (ACT costs 261ns + 0.833ns/elem regardless of func/bias/dtype, so
    wide tiles amortize the fixed cost; this engine is the bottleneck).
  - DVE negate (fp16 4x mode), DMA out [128, 2048] fp16 tiles.
"""

import os
import sys

import numpy as np
import ml_dtypes

try:
    import concourse  # noqa: F401
except ImportError:
    for _p in ("/opt/trn_rl_repo", os.path.expanduser("~/.axon_site/_ro/trn_rl_repo")):
        if os.path.isdir(_p) and _p not in sys.path:
            sys.path.insert(0, _p)

import concourse.bass as bass  # noqa: F401
import concourse.mybir as mybir
import concourse.tile as tile
from concourse import bacc
from concourse import bass_utils

N_CORES = 8
N, M, D = 8192, 8192, 128
R = N // N_CORES  # 1024 rows of z_anc per core
P = 128           # partitions
BANK = 512        # fp32 columns per PSUM bank
GRP = 2048        # columns per ACT/DVE/DMA group (4 banks)
MT = R // P       # 8 m-tiles per core
NG = M // GRP     # 4 n-groups

OUT_DT = mybir.dt.float16
_BF16 = ml_dtypes.bfloat16

_nc_cache = None


def _build():
    f32 = mybir.dt.float32
    bf16 = mybir.dt.bfloat16

    nc = bacc.Bacc("TRN2", debug=False, target_bir_lowering=False)
    aT = nc.dram_tensor("aT", [P, R], bf16, kind="ExternalInput").ap()
    aN = nc.dram_tensor("aN", [R, P], bf16, kind="ExternalInput").ap()
    bT = nc.dram_tensor("bT", [P, M], bf16, kind="ExternalInput").ap()
    out = nc.dram_tensor("out", [R, M], OUT_DT, kind="ExternalOutput").ap()

    with tile.TileContext(nc) as tc:
        with tc.tile_pool(name="consts", bufs=1) as consts:
            bT_sb = consts.tile([P, M], bf16)
            # group-0 columns first, in bank-sized sub-DMAs so the first
            # tile's squares/matmuls start as soon as sub-chunk 0 lands
            nc.sync.dma_start(out=bT_sb[:, 0:BANK], in_=bT[:, 0:BANK])
            aT_sb = consts.tile([P, R], bf16)
            nc.sync.dma_start(out=aT_sb, in_=aT)
            for j in range(1, GRP // BANK):
                sl = slice(j * BANK, (j + 1) * BANK)
                nc.sync.dma_start(out=bT_sb[:, sl], in_=bT[:, sl])
            # all eight aN row-tiles in ONE DMA via a 3D access pattern
            aN8 = consts.tile([P, R], bf16)  # [p, (t d)]
            aN_r = bass.AP(
                tensor=aN.tensor, offset=aN.offset,
                ap=[[D, P], [P * D, MT], [1, D]],
            )
            nc.sync.dma_start(
                out=aN8.rearrange("p (t d) -> p t d", d=D), in_=aN_r
            )

            w_b2 = consts.tile([P, P], bf16)   # -0.5, turns sq into -b2/2
            nc.vector.memset(w_b2, -0.5)
            scratch = consts.tile([P, BANK], bf16)  # PE warmup fodder
            nc.vector.memset(scratch, 0.001)
            junk = consts.tile([P, 8], f32)
            biasj = consts.tile([P, 1], f32)
            nc.vector.memset(biasj, 1.0)

            sqfull = consts.tile([P, M], bf16)  # bT^2, consumed by b2 matmuls
            a2c = consts.tile([P, MT], f32)     # ACT bias columns (= a2)

            with (
                tc.tile_pool(name="mm", bufs=2, space="PSUM") as mm_pool,
                tc.tile_pool(name="asq", bufs=2) as asq_pool,
                tc.tile_pool(name="o", bufs=6) as o_pool,
                tc.tile_pool(name="on", bufs=6) as on_pool,
            ):
                # preload the sqrt ACT table while DMAs are in flight
                nc.scalar.activation(
                    junk, scratch[:, 0:8], mybir.ActivationFunctionType.Sqrt,
                    bias=biasj[:, 0:1],
                )
                # PE warmup: HAM un-throttles after ~3.4us of activity, so
                # burn the DMA wait keeping the PE busy on scratch data
                wu = mm_pool.tile([P, GRP], f32, tag="ps")
                for k in range(12):
                    nc.tensor.matmul(
                        wu[:, (k % 4) * BANK:(k % 4) * BANK + BANK],
                        lhsT=scratch[:, 0:P], rhs=scratch,
                        start=True, stop=True,
                    )

                # squares for group 0 ahead of the a2 chain (DVE is in-order)
                for j in range(GRP // BANK):
                    sl = slice(j * BANK, (j + 1) * BANK)
                    nc.vector.tensor_mul(sqfull[:, sl], bT_sb[:, sl], bT_sb[:, sl])

                # ---- a2 on DVE only: square + free-dim reduce ----------
                for t in range(MT):
                    sl = slice(t * P, (t + 1) * P)
                    asq = asq_pool.tile([P, P], bf16, tag="asq")
                    nc.vector.tensor_mul(asq, aN8[:, sl], aN8[:, sl])
                    nc.vector.tensor_reduce(
                        a2c[:, t:t + 1], asq,
                        axis=mybir.AxisListType.X, op=mybir.AluOpType.add,
                    )

                # ---- main loop (n-group-major) -------------------------
                for g in range(NG):
                    # fetch the NEXT group's bT columns while computing this
                    if g + 1 < NG:
                        sl = slice((g + 1) * GRP, (g + 2) * GRP)
                        nc.sync.dma_start(out=bT_sb[:, sl], in_=bT[:, sl])
                    for j in range(GRP // BANK if g > 0 else 0):
                        c0 = g * GRP + j * BANK
                        sl = slice(c0, c0 + BANK)
                        nc.vector.tensor_mul(
                            sqfull[:, sl], bT_sb[:, sl], bT_sb[:, sl]
                        )
                    for t in range(MT):
                        ps = mm_pool.tile([P, GRP], f32, tag="ps")
                        for j in range(GRP // BANK):
                            c0 = g * GRP + j * BANK
                            nc.tensor.matmul(
                                ps[:, j * BANK:(j + 1) * BANK],
                                lhsT=aT_sb[:, t * P:(t + 1) * P],
                                rhs=bT_sb[:, c0:c0 + BANK],
                                start=True, stop=False,
                            )
                        for j in range(GRP // BANK):
                            c0 = g * GRP + j * BANK
                            nc.tensor.matmul(
                                ps[:, j * BANK:(j + 1) * BANK],
                                lhsT=w_b2,
                                rhs=sqfull[:, c0:c0 + BANK],
                                start=False, stop=True,
                            )
                        o = o_pool.tile([P, GRP], OUT_DT, tag="o")
                        nc.scalar.activation(
                            o, ps, mybir.ActivationFunctionType.Sqrt,
                            bias=a2c[:, t:t + 1], scale=-2.0,
                        )
                        on = on_pool.tile([P, GRP], OUT_DT, tag="on")
                        nc.vector.tensor_scalar_mul(on, o, -1.0)
                        nc.sync.dma_start(
                            out=out[t * P:(t + 1) * P, g * GRP:(g + 1) * GRP],
                            in_=on,
                        )

    nc.compile()
    return nc


def _get_nc():
    global _nc_cache
    if _nc_cache is None:
        _nc_cache = _build()
    return _nc_cache


def _in_maps(z_anc, z_pos_neg):
    za = np.asarray(z_anc, dtype=np.float32)
    zaT = np.ascontiguousarray(za.T)
    zbT = np.ascontiguousarray(np.asarray(z_pos_neg, dtype=np.float32).T)
    bT = zbT.astype(_BF16)
    maps = []
    for c in range(N_CORES):
        rows = slice(c * R, (c + 1) * R)
        aTc = np.ascontiguousarray(zaT[:, rows]).astype(_BF16)
        aNc = np.ascontiguousarray(za[rows, :]).astype(_BF16)
        maps.append({"aT": aTc, "aN": aNc, "bT": bT})
    return maps


def run(z_anc, z_pos_neg, **kwargs):
    """Run on hardware; returns (full_output, BassKernelResults)."""
    nc = _get_nc()
    res = bass_utils.run_bass_kernel_spmd(
        nc, _in_maps(z_anc, z_pos_neg), core_ids=list(range(N_CORES)), **kwargs
    )
    out = np.concatenate([r["out"] for r in res.results], axis=0)
    return out.astype(np.float32), res


def kernel(z_anc, z_pos_neg):
    out, _ = run(z_anc, z_pos_neg)
    return out
